# revision 1
# baseline (speedup 1.0000x reference)
"""Trainium2 Bass kernel for nn_CUDAOptimizedBKCore: diagonal Green's function
of a complex-shifted tridiagonal matrix via forward/backward continuant
recursions (theta/phi), data-parallel over the batch across 8 NeuronCores.

Self-contained: takes FULL inputs, shards B across cores, runs the Bass
program via run_bass_kernel_spmd, gathers the FULL output.
"""
import numpy as np

_CACHE = {}

import numpy as np

import numpy as np

import numpy as np
import concourse.bass as bass
import concourse.bacc as bacc
import concourse.tile as tile
from concourse import mybir

F32 = mybir.dt.float32
P = 128
RING = 16
CB = 8          # combine batch (phi'' values per batched combine)


def build_nc(b_core: int, n: int, f: int, n_cores: int = 8, gchunk: int = 8, loops: int = 1):
    """Build the Bacc program for one core's slice (b_core rows, n steps).

    Layout: partitions x f rows-per-partition. A tile holds a = he + d in
    (f, k) order. TH holds theta' history rows 0..n. phi'' lives in a
    16-slot ring; combines run batched (CB steps per instruction group),
    writing g into ring chunk tiles of `gchunk` k-columns DMA'd out as
    they complete.
    """
    assert b_core == P * f
    assert n % CB == 0 and gchunk % CB == 0
    nc = bacc.Bacc("TRN2", target_bir_lowering=False, debug=False, num_devices=n_cores)
    he = nc.dram_tensor("he", [b_core, n], F32, kind="ExternalInput").ap()
    dvec = nc.dram_tensor("dvec", [P, n], F32, kind="ExternalInput").ap()
    svf = nc.dram_tensor("svf", [P, n], F32, kind="ExternalInput").ap()
    svb = nc.dram_tensor("svb", [P, n], F32, kind="ExternalInput").ap()
    g = nc.dram_tensor("g", [b_core, 2 * n], F32, kind="ExternalOutput").ap()

    mult, add = mybir.AluOpType.mult, mybir.AluOpType.add

    with tile.TileContext(nc) as tc:
        with (
            tc.tile_pool(name="aux", bufs=1) as aux,
            tc.tile_pool(name="big", bufs=1) as big,
            tc.tile_pool(name="gring", bufs=2) as gring,
            tc.tile_pool(name="tmp", bufs=4) as tmp,
            tc.tile_pool(name="qtmp", bufs=2) as qtmp,
        ):
            d_t = aux.tile([P, n], F32)
            nc.sync.dma_start(out=d_t[:], in_=dvec)
            sf_t = aux.tile([P, n], F32)
            nc.sync.dma_start(out=sf_t[:], in_=svf)
            sb_t = aux.tile([P, n], F32)
            nc.sync.dma_start(out=sb_t[:], in_=svb)

            import contextlib
            loop_cm = tc.For_i(0, loops, 1) if loops > 1 else contextlib.nullcontext()
            with loop_cm:
                # A tile: (P, f, n) holds he then a = he + d in place
                A = big.tile([P, f, n], F32)
                nc.sync.dma_start(out=A[:], in_=he.rearrange("(p f) k -> p f k", p=P))
                d_b = d_t.unsqueeze(1).broadcast_to([P, f, n])
                nc.vector.tensor_add(out=A[:], in0=A[:], in1=d_b)

                TH = big.tile([P, n + 1, 2, f], F32)    # theta' history
                ring = big.tile([P, RING, 2, f], F32)   # phi'' ring

                def a_sl(k):
                    return A[:, :, k].unsqueeze(1).broadcast_to([P, 2, f])

                # init theta'_0 = (1, 0); theta'_1 = (1, a_0)
                nc.vector.memset(TH[:, 0, 0], 1.0)
                nc.gpsimd.memset(TH[:, 0, 1], 0.0)
                nc.vector.memset(TH[:, 1, 0], 1.0)
                nc.scalar.copy(out=TH[:, 1, 1], in_=A[:, :, 0])

                fh = f // 2

                def advance(t, c, c_swap, p, out, a_k, s_t):
                    """out = (c + s*p) -+ a (x) c_swap, run as two concurrent
                    row-half chains with mirrored engine assignment so each
                    engine's chain-stall holes are filled by the other half."""
                    for hf, sl in ((0, slice(0, fh)), (1, slice(fh, f))):
                        aH = A[:, sl, a_k]
                        cH, pH, oH = c[:, :, sl], p[:, :, sl], out[:, :, sl]
                        if hf == 0:
                            e_mr, e_mi, e_u, e_nr, e_ni = (
                                nc.gpsimd, nc.vector, nc.vector, nc.vector, nc.gpsimd)
                        else:
                            e_mr, e_mi, e_u, e_nr, e_ni = (
                                nc.vector, nc.gpsimd, nc.vector, nc.gpsimd, nc.vector)
                        m_r = tmp.tile([P, fh], F32, tag=f"mr{hf}", name=f"m_r{hf}")
                        e_mr.tensor_tensor(out=m_r[:], in0=cH[:, 1], in1=aH, op=mult)
                        m_i = tmp.tile([P, fh], F32, tag=f"mi{hf}", name=f"m_i{hf}")
                        e_mi.tensor_tensor(out=m_i[:], in0=cH[:, 0], in1=aH, op=mult)
                        u = tmp.tile([P, 2, fh], F32, tag=f"u{hf}", name=f"u{hf}")
                        e_u.scalar_tensor_tensor(
                            out=u[:], in0=pH, scalar=s_t, in1=cH, op0=mult, op1=add,
                        )
                        e_nr.tensor_sub(out=oH[:, 0], in0=u[:, 0], in1=m_r[:])
                        e_ni.tensor_add(out=oH[:, 1], in0=u[:, 1], in1=m_i[:])

                # ---- theta pass: t = 1..n-1 computes theta'_{t+1} ----
                for t in range(1, n):
                    advance(t, TH[:, t], TH[:, t, ::-1], TH[:, t - 1], TH[:, t + 1],
                            t, sf_t[:, t:t + 1])

                # ---- w = i / theta'_n -> ring slot 0 (phi''_0) ----
                dr, di = TH[:, n, 0], TH[:, n, 1]
                t1 = tmp.tile([P, f], F32, tag="w1", name="t1")
                nc.vector.tensor_mul(out=t1[:], in0=dr, in1=dr)
                t2 = tmp.tile([P, f], F32, tag="w2", name="t2")
                nc.gpsimd.tensor_mul(out=t2[:], in0=di, in1=di)
                nc.vector.tensor_add(out=t1[:], in0=t1[:], in1=t2[:])
                inv = tmp.tile([P, f], F32, tag="w3", name="inv")
                nc.vector.reciprocal(out=inv[:], in_=t1[:])
                nc.vector.tensor_mul(out=ring[:, 0, 0], in0=di, in1=inv[:])
                nc.vector.tensor_mul(out=ring[:, 0, 1], in0=dr, in1=inv[:])

                # phi''_1 = w + (a_{n-1} (x) w_swap) signs (-,+)  -> ring slot 1
                m0 = tmp.tile([P, 2, f], F32, tag="m", name="m0")
                nc.gpsimd.tensor_tensor(
                    out=m0[:], in0=ring[:, 0, ::-1], in1=a_sl(n - 1), op=mult
                )
                nc.vector.tensor_sub(out=ring[:, 1, 0], in0=ring[:, 0, 0], in1=m0[:, 0])
                nc.vector.tensor_add(out=ring[:, 1, 1], in0=ring[:, 0, 1], in1=m0[:, 1])

                g4 = g.rearrange("(p f) (k c) -> p f k c", p=P, c=2)
                chunk_t = {}

                def combine_batch(b):
                    """G_{n-1-m} = theta'_{n-1-m} (x) phi''_m for m in [CB*b, CB*b+CB)"""
                    m0i = CB * b
                    s0 = m0i % RING
                    hi = (n - 1) - m0i            # highest theta row in batch
                    ci = (hi - CB + 1) // gchunk
                    if ci not in chunk_t:
                        chunk_t[ci] = gring.tile(
                            [P, f, gchunk, 2], F32, tag="g", name=f"gchunk{ci}"
                        )
                    gc = chunk_t[ci]
                    xs = None if hi - CB < 0 else hi - CB
                    X = TH[:, hi:xs:-1]                   # (P, CB, 2, f) rows desc
                    Y = ring[:, s0:s0 + CB]               # (P, CB, 2, f)
                    Ys = ring[:, s0:s0 + CB, ::-1]
                    h = CB // 2
                    q1 = qtmp.tile([P, CB, 2, f], F32, tag="q1", name="q1")
                    nc.gpsimd.tensor_tensor(out=q1[:, :h], in0=X[:, :h], in1=Y[:, :h], op=mult)
                    nc.vector.tensor_tensor(out=q1[:, h:], in0=X[:, h:], in1=Y[:, h:], op=mult)
                    q2 = qtmp.tile([P, CB, 2, f], F32, tag="q2", name="q2")
                    nc.gpsimd.tensor_tensor(out=q2[:, :h], in0=X[:, :h], in1=Ys[:, :h], op=mult)
                    nc.vector.tensor_tensor(out=q2[:, h:], in0=X[:, h:], in1=Ys[:, h:], op=mult)
                    jhi = hi - gchunk * ci
                    js = None if jhi - CB < 0 else jhi - CB
                    og_r = gc[:, :, jhi:js:-1, 0].transpose([0, 2, 1])
                    og_i = gc[:, :, jhi:js:-1, 1].transpose([0, 2, 1])
                    nc.vector.tensor_sub(out=og_r[:, :h], in0=q1[:, :h, 0], in1=q1[:, :h, 1])
                    nc.vector.tensor_sub(out=og_r[:, h:], in0=q1[:, h:, 0], in1=q1[:, h:, 1])
                    nc.gpsimd.tensor_add(out=og_i[:, :h], in0=q2[:, :h, 0], in1=q2[:, :h, 1])
                    nc.vector.tensor_add(out=og_i[:, h:], in0=q2[:, h:, 0], in1=q2[:, h:, 1])
                    if jhi - CB + 1 == 0:                 # chunk complete -> DMA out
                        k0 = gchunk * ci
                        nc.sync.dma_start(
                            out=g4[:, :, k0:k0 + gchunk, :], in_=gc[:]
                        )
                        del chunk_t[ci]

                # ---- phi pass: t = 1..n-2 computes phi''_{t+1} ----
                for t in range(1, n - 1):
                    advance(t, ring[:, t % RING], ring[:, t % RING, ::-1],
                            ring[:, (t - 1) % RING], ring[:, (t + 1) % RING],
                            n - 1 - t, sb_t[:, t:t + 1])
                    if (t + 2) % CB == 0:                 # phi''_{t+1} closes batch
                        combine_batch((t + 2) // CB - 1)

    nc.compile()
    return nc


def make_aux(h0_diag: np.ndarray, h0_sub: np.ndarray, h0_super: np.ndarray, n: int):
    s = (h0_super * h0_sub).astype(np.float32)          # (n-1,)
    d = h0_diag.astype(np.float32)                      # (n,)
    svf = np.zeros(n, np.float32)
    svf[1:] = s                                          # svf[t] = s[t-1]
    svb = np.zeros(n, np.float32)
    svb[1:n - 1] = s[::-1][:n - 2]                       # svb[t] = s[n-1-t]
    dvec = np.broadcast_to(d, (P, n)).copy()
    svf = np.broadcast_to(svf, (P, n)).copy()
    svb = np.broadcast_to(svb, (P, n)).copy()
    return dvec, svf, svb




def _get_nc(b_core, n, f, n_cores):
    key = (b_core, n, f, n_cores)
    if key not in _CACHE:
        _CACHE[key] = build_nc(b_core, n, f, n_cores=n_cores)
    return _CACHE[key]


def kernel(he_diag, h0_diag, h0_sub, h0_super):
    from concourse.bass_utils import run_bass_kernel_spmd

    he_diag = np.ascontiguousarray(np.asarray(he_diag, dtype=np.float32))
    B, n = he_diag.shape
    n_cores = 8
    assert B % n_cores == 0
    b_core = B // n_cores
    assert b_core % P == 0
    f = b_core // P

    dvec, svf, svb = make_aux(
        np.asarray(h0_diag), np.asarray(h0_sub), np.asarray(h0_super), n
    )
    nc = _get_nc(b_core, n, f, n_cores)
    in_maps = [
        {"he": he_diag[c * b_core:(c + 1) * b_core],
         "dvec": dvec, "svf": svf, "svb": svb}
        for c in range(n_cores)
    ]
    res = run_bass_kernel_spmd(nc, in_maps, list(range(n_cores)))
    out = np.concatenate(
        [res.results[c]["g"].reshape(b_core, n, 2) for c in range(n_cores)], axis=0
    )
    return out

